# revision 2
# baseline (speedup 1.0000x reference)
"""GSAPool pairwise-distance + mean-threshold adjacency kernel for TRN2.

dist[b,i,j] = sqrt(||x_i||^2 + ||y_j||^2 - 2 x_i.y_j), mask = dist <= mean_b(dist)

Sharding: pure data-parallel over batch b: 64 samples -> 8 cores x 8 samples.

v5 design notes:
  - all big matmuls in float32r (1 cyc/row at N=512 vs fp32's 4): HW-verified
    bit-identical results to fp32 matmul on this silicon.
  - yy row-add as a rank-2 hi/lo f32r matmul (hi = 7-mantissa-bit truncation
    via u32 masking, lo = remainder): exact despite the ~12-bit truncation of
    f32r moving operands. The [16,128] -> [2,1024] reshape bounces through a
    DRAM scratch because SBUF-side partition-dim splits in DMA rearranges do
    not follow einops factor order (HW-verified t-block permutation bug).
  - dist output written as fp16 (halves dist HBM traffic); mask compare runs
    on the fp32 dist kept in SBUF (fp16 compare would flip ~0.2% of mask).
  - software pipelining with fine-grained interleave: the front phase of
    sample s+1 (loads, norms, yy chain, PE transposes) is emitted chunk-wise
    BETWEEN the main matmul groups of sample s, so the in-order PE queue
    always has real matmuls in flight (keeps the HAM clock-gate at 8/8 --
    transpose-mode ops do not count as PE activity) and the cross-engine yy
    chain has a full sample period of latency slack.
"""

import numpy as np
from contextlib import ExitStack

import concourse.bass as bass
import concourse.tile as tile
from concourse import bacc, mybir
from concourse.bass_utils import run_bass_kernel_spmd
from concourse.masks import make_identity

B = 64
M = 1024
N = 1024
D = 256
P = 128
MT = M // P        # 8 m-tiles
KT = D // P        # 2 k-tiles
NCORES = 8
S = B // NCORES    # 8 samples per core
F32 = mybir.dt.float32
F32R = mybir.dt.float32r
F16 = mybir.dt.float16
U8 = mybir.dt.uint8
U32 = mybir.dt.uint32
ALU = mybir.AluOpType
ACTF = mybir.ActivationFunctionType


def build_body(ctx, tc, x_d, y_d, dist_d, mask_d, yysc_d, n_samples):
    nc = tc.nc

    const_pool = ctx.enter_context(tc.tile_pool(name="const", bufs=1))
    ident_f = const_pool.tile([P, P], F32)
    make_identity(nc, ident_f[:])
    ident_r = const_pool.tile([P, P], F32R)
    nc.vector.tensor_copy(ident_r[:], ident_f[:])
    ones2_f = const_pool.tile([2, P], F32)
    nc.gpsimd.memset(ones2_f[:, :], 1.0)
    ones2_r = const_pool.tile([2, P], F32R)
    nc.vector.tensor_copy(ones2_r[:], ones2_f[:])
    ones_col = const_pool.tile([P, 8], F32)
    nc.gpsimd.memset(ones_col[:], 1.0)
    ones_row2 = const_pool.tile([2, P], F32)
    nc.gpsimd.memset(ones_row2[:, :], 0.0)
    nc.gpsimd.memset(ones_row2[0:1, :], 1.0)

    nat_pool = ctx.enter_context(tc.tile_pool(name="nat", bufs=2))
    tr_pool = ctx.enter_context(tc.tile_pool(name="tr", bufs=2))
    dist_pool = ctx.enter_context(tc.tile_pool(name="dist", bufs=10))
    d16_pool = ctx.enter_context(tc.tile_pool(name="d16", bufs=3))
    mask_pool = ctx.enter_context(tc.tile_pool(name="mask", bufs=2))
    small_pool = ctx.enter_context(tc.tile_pool(name="small", bufs=3))
    psum_tr = ctx.enter_context(tc.tile_pool(name="psum_tr", bufs=2, space="PSUM"))
    psum_d2 = ctx.enter_context(tc.tile_pool(name="psum_d2", bufs=2, space="PSUM"))
    psum_sm = ctx.enter_context(tc.tile_pool(name="psum_sm", bufs=2, space="PSUM"))

    def front_chunks(s):
        """Generator: chunk 0 = loads+norms+yy chain; chunks 1..8 = transpose
        batches (4 PE transposes + 1 DVE copy each)."""
        x_nat = nat_pool.tile([P, MT * D], F32R, tag="x_nat")
        nc.sync.dma_start(
            out=x_nat.rearrange("p (t d) -> p t d", t=MT),
            in_=x_d[s].bitcast(F32R).rearrange("(t p) d -> p t d", p=P),
        )
        y_nat = nat_pool.tile([P, MT * D], F32R, tag="y_nat")
        nc.sync.dma_start(
            out=y_nat.rearrange("p (t d) -> p t d", t=MT),
            in_=y_d[s].bitcast(F32R).rearrange("(t p) d -> p t d", p=P),
        )

        xx8 = small_pool.tile([P, MT], F32, tag="xx8")
        yy8 = small_pool.tile([P, MT], F32, tag="yy8")
        for t in range(MT):
            sq_scratch = small_pool.tile([P, D], F32, tag="sq_scratch")
            nc.scalar.activation(
                sq_scratch[:],
                y_nat[:, t * D:(t + 1) * D].bitcast(F32),
                ACTF.Square,
                bias=0.0,
                scale=1.0,
                accum_out=yy8[:, t:t + 1],
            )
        # yy -> rank-2 hi/lo rows; hi in cols 0:8, lo in cols 8:16; transpose
        # puts hi on psum partitions 0-7, lo on 8-15; reshape via DRAM bounce.
        yy8m = small_pool.tile([P, MT], F32, tag="yy8m")
        nc.vector.tensor_scalar_mul(yy8m[:], yy8[:], -0.5)
        yyhl_f = small_pool.tile([P, 2 * MT], F32, tag="yyhl_f")
        nc.vector.tensor_scalar(
            yyhl_f[:, 0:MT].bitcast(U32), yy8m[:].bitcast(U32), 0xFFFF0000, None,
            ALU.bitwise_and,
        )
        nc.vector.tensor_tensor(
            yyhl_f[:, MT:2 * MT], yy8m[:], yyhl_f[:, 0:MT], ALU.subtract
        )
        yyhl_r = small_pool.tile([P, 2 * MT], F32R, tag="yyhl_r")
        nc.vector.tensor_copy(yyhl_r[:], yyhl_f[:])
        p_hl = psum_sm.tile([2 * MT, P], F32R, tag="sm")
        nc.tensor.transpose(p_hl[:], yyhl_r[:], ident_r[:])
        yyT2 = small_pool.tile([2 * MT, P], F32R, tag="yyT2")
        nc.vector.tensor_copy(yyT2[:], p_hl[:])
        nc.sync.dma_start(out=yysc_d[s].bitcast(F32R), in_=yyT2[:])
        yyrow2 = small_pool.tile([2, N], F32R, tag="yyrow2")
        nc.sync.dma_start(
            out=yyrow2[:],
            in_=yysc_d[s].bitcast(F32R).rearrange("(two t) p -> two (t p)", two=2),
        )

        for t in range(MT):
            sq_scratch = small_pool.tile([P, D], F32, tag="sq_scratch")
            nc.scalar.activation(
                sq_scratch[:],
                x_nat[:, t * D:(t + 1) * D].bitcast(F32),
                ACTF.Square,
                bias=0.0,
                scale=1.0,
                accum_out=xx8[:, t:t + 1],
            )

        xT = tr_pool.tile([P, KT * M], F32R, tag="xT")
        yT = tr_pool.tile([P, KT * N], F32R, tag="yT")
        state = (xx8, yyrow2, xT, yT)
        yield state

        for src, dstT in ((x_nat, xT), (y_nat, yT)):
            for kt in range(KT):
                for tq in range(2):
                    ptile = psum_tr.tile([P, 512], F32R, tag="tr")
                    for j in range(4):
                        t = tq * 4 + j
                        nc.tensor.transpose(
                            ptile[:, j * P:(j + 1) * P],
                            src[:, t * D + kt * P: t * D + kt * P + P],
                            ident_r[:],
                        )
                    nc.vector.tensor_copy(
                        dstT[:, kt * M + tq * 512: kt * M + (tq + 1) * 512],
                        ptile[:],
                    )
                    yield state

    def main_chunks(s, front):
        """Generator: chunks 0..7 = one m-tile matmul group + sqrt + fp16 out;
        final chunk = mean + mask + stores."""
        xx8, yyrow2, xT, yT = front
        rs = small_pool.tile([P, MT], F32, tag="rs")
        dist_tiles = []
        for i in range(MT):
            pd = psum_d2.tile([P, N], F32, tag="d2")
            for nh in range(2):
                for kt in range(KT):
                    nc.tensor.matmul(
                        pd[:, nh * 512:(nh + 1) * 512],
                        xT[:, kt * M + i * P: kt * M + (i + 1) * P],
                        yT[:, kt * N + nh * 512: kt * N + nh * 512 + 512],
                        start=(kt == 0),
                        stop=False,
                    )
                nc.tensor.matmul(
                    pd[:, nh * 512:(nh + 1) * 512],
                    ones2_r[:],
                    yyrow2[:, nh * 512:(nh + 1) * 512],
                    start=False,
                    stop=True,
                )
            dt_tile = dist_pool.tile([P, N], F32, tag="dist")
            nc.scalar.activation(
                dt_tile[:],
                pd[:],
                ACTF.Sqrt,
                bias=xx8[:, i:i + 1],
                scale=-2.0,
                accum_out=rs[:, i:i + 1],
            )
            d16 = d16_pool.tile([P, N], F16, tag="d16")
            nc.vector.tensor_copy(d16[:], dt_tile[:])
            nc.sync.dma_start(out=dist_d[s, i * P:(i + 1) * P, :], in_=d16[:])
            dist_tiles.append(dt_tile)
            yield

        ptot = psum_sm.tile([8, MT], F32, tag="sm")
        nc.tensor.matmul(ptot[:], ones_col[:], rs[:], start=True, stop=True)
        tot = small_pool.tile([2, 8], F32, tag="tot")
        nc.gpsimd.memset(tot[:, :], 0.0)
        nc.vector.tensor_reduce(
            out=tot[0:1, 0:1], in_=ptot[0:1, :], axis=mybir.AxisListType.X, op=ALU.add
        )
        pavg = psum_sm.tile([P, 8], F32, tag="sm")
        nc.tensor.matmul(pavg[:], ones_row2[:], tot[:], start=True, stop=True)
        avg = small_pool.tile([P, 1], F32, tag="avg")
        nc.scalar.activation(
            avg[:], pavg[:, 0:1], ACTF.Copy, bias=0.0, scale=1.0 / float(M * N)
        )

        mask_all = mask_pool.tile([P, MT * N], U8, tag="mask")
        for i in range(MT):
            nc.vector.tensor_scalar(
                mask_all[:, i * N:(i + 1) * N],
                dist_tiles[i][:],
                avg[:, 0:1],
                None,
                ALU.is_le,
            )
        nc.sync.dma_start(
            out=mask_d[s].rearrange("(t p) n -> p t n", p=P),
            in_=mask_all.rearrange("p (t n) -> p t n", t=MT),
        )
        yield

    # ---- coarse pipelining: full front(s+1) before main(s) ----
    def run_front(s):
        g = front_chunks(s)
        st = next(g)
        for _ in g:
            pass
        return st

    fronts = {0: run_front(0)}
    for s in range(n_samples):
        if s + 1 < n_samples:
            fronts[s + 1] = run_front(s + 1)
        for _ in main_chunks(s, fronts.pop(s)):
            pass


def build_program(n_samples=S, num_devices=NCORES):
    nc = bacc.Bacc(
        "TRN2", target_bir_lowering=False, debug=False, num_devices=num_devices
    )
    x_d = nc.dram_tensor("x", [n_samples, M, D], F32, kind="ExternalInput").ap()
    y_d = nc.dram_tensor("y", [n_samples, N, D], F32, kind="ExternalInput").ap()
    dist_d = nc.dram_tensor("dist", [n_samples, M, N], F16, kind="ExternalOutput").ap()
    mask_d = nc.dram_tensor("mask", [n_samples, M, N], U8, kind="ExternalOutput").ap()
    yysc_d = nc.dram_tensor("yysc", [n_samples, 2 * MT, P], F32, kind="Internal").ap()
    with tile.TileContext(nc) as tc:
        with ExitStack() as ctx:
            build_body(ctx, tc, x_d, y_d, dist_d, mask_d, yysc_d, n_samples)
    nc.compile()
    return nc


_nc_cache = None
_F16_LUT = None


def _f16_to_f32(a):
    global _F16_LUT
    if _F16_LUT is None:
        _F16_LUT = np.arange(65536, dtype=np.uint16).view(np.float16).astype(np.float32)
    return _F16_LUT[a.view(np.uint16)]


def _get_nc():
    global _nc_cache
    if _nc_cache is None:
        _nc_cache = build_program()
    return _nc_cache


def kernel(x, y):
    x = np.ascontiguousarray(np.asarray(x), dtype=np.float32).reshape(B, M, D)
    y = np.ascontiguousarray(np.asarray(y), dtype=np.float32).reshape(B, N, D)
    nc = _get_nc()
    in_maps = [
        {
            "x": np.ascontiguousarray(x[c * S:(c + 1) * S]),
            "y": np.ascontiguousarray(y[c * S:(c + 1) * S]),
        }
        for c in range(NCORES)
    ]
    res = run_bass_kernel_spmd(nc, in_maps, list(range(NCORES)))
    dist = np.concatenate(
        [_f16_to_f32(res.results[c]["dist"]) for c in range(NCORES)], axis=0
    )
    mask = np.concatenate([res.results[c]["mask"] for c in range(NCORES)], axis=0)
    return dist, mask != 0


# revision 3
# speedup vs baseline: 1.1541x; 1.1541x over previous
"""GSAPool pairwise-distance + mean-threshold adjacency kernel for TRN2.

dist[b,i,j] = sqrt(||x_i||^2 + ||y_j||^2 - 2 x_i.y_j), mask = dist <= mean_b(dist)

Sharding: pure data-parallel over batch b: 64 samples -> 8 cores x 8 samples.

v5 design notes:
  - all big matmuls in float32r (1 cyc/row at N=512 vs fp32's 4): HW-verified
    bit-identical results to fp32 matmul on this silicon.
  - yy row-add as a rank-2 hi/lo f32r matmul (hi = 7-mantissa-bit truncation
    via u32 masking, lo = remainder): exact despite the ~12-bit truncation of
    f32r moving operands. The [16,128] -> [2,1024] reshape bounces through a
    DRAM scratch because SBUF-side partition-dim splits in DMA rearranges do
    not follow einops factor order (HW-verified t-block permutation bug).
  - dist output written as fp16 (halves dist HBM traffic); mask compare runs
    on the fp32 dist kept in SBUF (fp16 compare would flip ~0.2% of mask).
  - software pipelining with fine-grained interleave: the front phase of
    sample s+1 (loads, norms, yy chain, PE transposes) is emitted chunk-wise
    BETWEEN the main matmul groups of sample s, so the in-order PE queue
    always has real matmuls in flight (keeps the HAM clock-gate at 8/8 --
    transpose-mode ops do not count as PE activity) and the cross-engine yy
    chain has a full sample period of latency slack.
"""

import numpy as np
from contextlib import ExitStack

import concourse.bass as bass
import concourse.tile as tile
from concourse import bacc, mybir
from concourse.bass_utils import run_bass_kernel_spmd
from concourse.masks import make_identity

B = 64
M = 1024
N = 1024
D = 256
P = 128
MT = M // P        # 8 m-tiles
KT = D // P        # 2 k-tiles
NCORES = 8
S = B // NCORES    # 8 samples per core
F32 = mybir.dt.float32
F32R = mybir.dt.float32r
F16 = mybir.dt.float16
U8 = mybir.dt.uint8
U32 = mybir.dt.uint32
ALU = mybir.AluOpType
ACTF = mybir.ActivationFunctionType


def build_body(ctx, tc, x_d, y_d, dist_d, mask_d, yysc_d, n_samples):
    nc = tc.nc

    const_pool = ctx.enter_context(tc.tile_pool(name="const", bufs=1))
    ident_f = const_pool.tile([P, P], F32)
    make_identity(nc, ident_f[:])
    ident_r = const_pool.tile([P, P], F32R)
    nc.vector.tensor_copy(ident_r[:], ident_f[:])
    ones2_f = const_pool.tile([2, P], F32)
    nc.gpsimd.memset(ones2_f[:, :], 1.0)
    ones2_r = const_pool.tile([2, P], F32R)
    nc.vector.tensor_copy(ones2_r[:], ones2_f[:])
    ones_col = const_pool.tile([P, 8], F32)
    nc.gpsimd.memset(ones_col[:], 1.0)
    ones_row2 = const_pool.tile([2, P], F32)
    nc.gpsimd.memset(ones_row2[:, :], 0.0)
    nc.gpsimd.memset(ones_row2[0:1, :], 1.0)

    nat_pool = ctx.enter_context(tc.tile_pool(name="nat", bufs=2))
    tr_pool = ctx.enter_context(tc.tile_pool(name="tr", bufs=2))
    dist_pool = ctx.enter_context(tc.tile_pool(name="dist", bufs=17))
    d16_pool = ctx.enter_context(tc.tile_pool(name="d16", bufs=4))
    mask_pool = ctx.enter_context(tc.tile_pool(name="mask", bufs=2))
    small_pool = ctx.enter_context(tc.tile_pool(name="small", bufs=3))
    psum_tr = ctx.enter_context(tc.tile_pool(name="psum_tr", bufs=2, space="PSUM"))
    psum_d2 = ctx.enter_context(tc.tile_pool(name="psum_d2", bufs=2, space="PSUM"))
    psum_sm = ctx.enter_context(tc.tile_pool(name="psum_sm", bufs=2, space="PSUM"))

    def front_chunks(s):
        """Generator: chunk 0 = loads+norms+yy chain; chunks 1..8 = transpose
        batches (4 PE transposes + 1 DVE copy each)."""
        x_nat = nat_pool.tile([P, MT * D], F32R, tag="x_nat")
        nc.sync.dma_start(
            out=x_nat.rearrange("p (t d) -> p t d", t=MT),
            in_=x_d[s].bitcast(F32R).rearrange("(t p) d -> p t d", p=P),
        )
        y_nat = nat_pool.tile([P, MT * D], F32R, tag="y_nat")
        nc.sync.dma_start(
            out=y_nat.rearrange("p (t d) -> p t d", t=MT),
            in_=y_d[s].bitcast(F32R).rearrange("(t p) d -> p t d", p=P),
        )

        xx8 = small_pool.tile([P, MT], F32, tag="xx8")
        yy8 = small_pool.tile([P, MT], F32, tag="yy8")
        for t in range(MT):
            sq_scratch = small_pool.tile([P, D], F32, tag="sq_scratch")
            nc.scalar.activation(
                sq_scratch[:],
                y_nat[:, t * D:(t + 1) * D].bitcast(F32),
                ACTF.Square,
                bias=0.0,
                scale=1.0,
                accum_out=yy8[:, t:t + 1],
            )
        # yy -> rank-2 hi/lo rows; hi in cols 0:8, lo in cols 8:16; transpose
        # puts hi on psum partitions 0-7, lo on 8-15; reshape via DRAM bounce.
        yy8m = small_pool.tile([P, MT], F32, tag="yy8m")
        nc.vector.tensor_scalar_mul(yy8m[:], yy8[:], -0.5)
        yyhl_f = small_pool.tile([P, 2 * MT], F32, tag="yyhl_f")
        nc.vector.tensor_scalar(
            yyhl_f[:, 0:MT].bitcast(U32), yy8m[:].bitcast(U32), 0xFFFF0000, None,
            ALU.bitwise_and,
        )
        nc.vector.tensor_tensor(
            yyhl_f[:, MT:2 * MT], yy8m[:], yyhl_f[:, 0:MT], ALU.subtract
        )
        yyhl_r = small_pool.tile([P, 2 * MT], F32R, tag="yyhl_r")
        nc.vector.tensor_copy(yyhl_r[:], yyhl_f[:])
        p_hl = psum_sm.tile([2 * MT, P], F32R, tag="sm")
        nc.tensor.transpose(p_hl[:], yyhl_r[:], ident_r[:])
        yyT2 = small_pool.tile([2 * MT, P], F32R, tag="yyT2")
        nc.vector.tensor_copy(yyT2[:], p_hl[:])
        nc.sync.dma_start(out=yysc_d[s].bitcast(F32R), in_=yyT2[:])
        yyrow2 = small_pool.tile([2, N], F32R, tag="yyrow2")
        nc.sync.dma_start(
            out=yyrow2[:],
            in_=yysc_d[s].bitcast(F32R).rearrange("(two t) p -> two (t p)", two=2),
        )

        for t in range(MT):
            sq_scratch = small_pool.tile([P, D], F32, tag="sq_scratch")
            nc.scalar.activation(
                sq_scratch[:],
                x_nat[:, t * D:(t + 1) * D].bitcast(F32),
                ACTF.Square,
                bias=0.0,
                scale=1.0,
                accum_out=xx8[:, t:t + 1],
            )

        xT = tr_pool.tile([P, KT * M], F32R, tag="xT")
        yT = tr_pool.tile([P, KT * N], F32R, tag="yT")
        state = (xx8, yyrow2, xT, yT)
        yield state

        for src, dstT in ((x_nat, xT), (y_nat, yT)):
            for kt in range(KT):
                for tq in range(2):
                    ptile = psum_tr.tile([P, 512], F32R, tag="tr")
                    for j in range(4):
                        t = tq * 4 + j
                        nc.tensor.transpose(
                            ptile[:, j * P:(j + 1) * P],
                            src[:, t * D + kt * P: t * D + kt * P + P],
                            ident_r[:],
                        )
                    nc.vector.tensor_copy(
                        dstT[:, kt * M + tq * 512: kt * M + (tq + 1) * 512],
                        ptile[:],
                    )
                    yield state

    def main_chunks(s, front):
        """Generator: chunks 0..7 = one m-tile matmul group + sqrt + fp16 out;
        final chunk = mean + mask + stores."""
        xx8, yyrow2, xT, yT = front
        rs = small_pool.tile([P, MT], F32, tag="rs")
        dist_tiles = []
        for i in range(MT):
            pd = psum_d2.tile([P, N], F32, tag="d2")
            for nh in range(2):
                for kt in range(KT):
                    nc.tensor.matmul(
                        pd[:, nh * 512:(nh + 1) * 512],
                        xT[:, kt * M + i * P: kt * M + (i + 1) * P],
                        yT[:, kt * N + nh * 512: kt * N + nh * 512 + 512],
                        start=(kt == 0),
                        stop=False,
                    )
                nc.tensor.matmul(
                    pd[:, nh * 512:(nh + 1) * 512],
                    ones2_r[:],
                    yyrow2[:, nh * 512:(nh + 1) * 512],
                    start=False,
                    stop=True,
                )
            dt_tile = dist_pool.tile([P, N], F32, tag="dist")
            nc.scalar.activation(
                dt_tile[:],
                pd[:],
                ACTF.Sqrt,
                bias=xx8[:, i:i + 1],
                scale=-2.0,
                accum_out=rs[:, i:i + 1],
            )
            d16 = d16_pool.tile([P, N], F16, tag="d16")
            nc.vector.tensor_copy(d16[:], dt_tile[:])
            nc.sync.dma_start(out=dist_d[s, i * P:(i + 1) * P, :], in_=d16[:])
            dist_tiles.append(dt_tile)
            yield

        ptot = psum_sm.tile([8, MT], F32, tag="sm")
        nc.tensor.matmul(ptot[:], ones_col[:], rs[:], start=True, stop=True)
        tot = small_pool.tile([2, 8], F32, tag="tot")
        nc.gpsimd.memset(tot[:, :], 0.0)
        nc.vector.tensor_reduce(
            out=tot[0:1, 0:1], in_=ptot[0:1, :], axis=mybir.AxisListType.X, op=ALU.add
        )
        pavg = psum_sm.tile([P, 8], F32, tag="sm")
        nc.tensor.matmul(pavg[:], ones_row2[:], tot[:], start=True, stop=True)
        avg = small_pool.tile([P, 1], F32, tag="avg")
        nc.scalar.activation(
            avg[:], pavg[:, 0:1], ACTF.Copy, bias=0.0, scale=1.0 / float(M * N)
        )

        mask_all = mask_pool.tile([P, MT * N], U8, tag="mask")
        for i in range(MT):
            nc.vector.tensor_scalar(
                mask_all[:, i * N:(i + 1) * N],
                dist_tiles[i][:],
                avg[:, 0:1],
                None,
                ALU.is_le,
            )
        nc.sync.dma_start(
            out=mask_d[s].rearrange("(t p) n -> p t n", p=P),
            in_=mask_all.rearrange("p (t n) -> p t n", t=MT),
        )
        yield

    # ---- coarse pipelining: full front(s+1) before main(s) ----
    def run_front(s):
        g = front_chunks(s)
        st = next(g)
        for _ in g:
            pass
        return st

    fronts = {0: run_front(0)}
    for s in range(n_samples):
        if s + 1 < n_samples:
            fronts[s + 1] = run_front(s + 1)
        for _ in main_chunks(s, fronts.pop(s)):
            pass


def build_program(n_samples=S, num_devices=NCORES):
    nc = bacc.Bacc(
        "TRN2", target_bir_lowering=False, debug=False, num_devices=num_devices
    )
    x_d = nc.dram_tensor("x", [n_samples, M, D], F32, kind="ExternalInput").ap()
    y_d = nc.dram_tensor("y", [n_samples, N, D], F32, kind="ExternalInput").ap()
    dist_d = nc.dram_tensor("dist", [n_samples, M, N], F16, kind="ExternalOutput").ap()
    mask_d = nc.dram_tensor("mask", [n_samples, M, N], U8, kind="ExternalOutput").ap()
    yysc_d = nc.dram_tensor("yysc", [n_samples, 2 * MT, P], F32, kind="Internal").ap()
    with tile.TileContext(nc) as tc:
        with ExitStack() as ctx:
            build_body(ctx, tc, x_d, y_d, dist_d, mask_d, yysc_d, n_samples)
    nc.compile()
    return nc


_nc_cache = None
_F16_LUT = None


def _f16_to_f32(a):
    global _F16_LUT
    if _F16_LUT is None:
        _F16_LUT = np.arange(65536, dtype=np.uint16).view(np.float16).astype(np.float32)
    return _F16_LUT[a.view(np.uint16)]


def _get_nc():
    global _nc_cache
    if _nc_cache is None:
        _nc_cache = build_program()
    return _nc_cache


def kernel(x, y):
    x = np.ascontiguousarray(np.asarray(x), dtype=np.float32).reshape(B, M, D)
    y = np.ascontiguousarray(np.asarray(y), dtype=np.float32).reshape(B, N, D)
    nc = _get_nc()
    in_maps = [
        {
            "x": np.ascontiguousarray(x[c * S:(c + 1) * S]),
            "y": np.ascontiguousarray(y[c * S:(c + 1) * S]),
        }
        for c in range(NCORES)
    ]
    res = run_bass_kernel_spmd(nc, in_maps, list(range(NCORES)))
    dist = np.concatenate(
        [_f16_to_f32(res.results[c]["dist"]) for c in range(NCORES)], axis=0
    )
    mask = np.concatenate([res.results[c]["mask"] for c in range(NCORES)], axis=0)
    return dist, mask != 0
